# Initial kernel scaffold
#
"""Trainium2 8-core kernel for the LSTM seq2seq + attention + vocab-projection model.

Strategy:
  - LSTM recurrence: tensor-parallel over the gate dimension. Core m owns
    h-slice [m*128:(m+1)*128) and computes the 4 gate rows for that slice
    (packed on host in order [f, i, o, g], 128 rows each). After each step the
    h-slices are AllGathered (bf16, 8KB) so every core holds the full h for
    the next step's matmul. Cell state c stays sharded; it is never exchanged.
  - The additive attention collapses: softmax over (dec@wd + enc@we + b) along
    the encoder axis is independent of the decoder position, so the context
    vector is per-batch constant. ctx[b] = softmax_e(enc_out[b]@we) @ enc_out[b].
  - Final projection is vocab-sharded: core m computes rows [m*4000,(m+1)*4000)
    (padded to 4096) of  out = dec_out @ fc_w[:, :H].T + (ctx @ fc_w[:, H:].T
    + fc_b).  The fc work is emitted as background chunks interleaved into the
    decoder phase so it runs in PE gaps while steps wait on the AllGather.
Token index convention: tau = t*16 + b  (time-major, batch inner).
"""

import os
import sys

for _p in ("/opt/trn_rl_repo", "/root/.axon_site/_ro/trn_rl_repo"):
    if os.path.isdir(_p) and _p not in sys.path:
        sys.path.insert(0, _p)

import numpy as np
import ml_dtypes

import concourse.bass as bass
import concourse.bacc as bacc
import concourse.tile as tile
from concourse import mybir
from concourse.bass_utils import run_bass_kernel_spmd

BF16 = ml_dtypes.bfloat16
DT = mybir.dt
AF = mybir.ActivationFunctionType
ALU = mybir.AluOpType

B = 16
T = 128          # both encoder and decoder length
H = 1024
V = 32000
NC = 8
HL = H // NC     # 128  h-slice per core
KT = H // 128    # 8    K tiles of the hidden dim
T2 = B * T       # 2048 tokens
VL = V // NC     # 4000 real vocab rows per core
VLP = 4096       # padded vocab rows per core
MT = VLP // 128  # 32   vocab M-tiles per core
# gate order on device: [f, i, o, g]; torch rows are [i, f, g, o]
GATE_SRC = (1, 0, 3, 2)


def _bcast(ap, dim, count):
    """Insert a [step=0, count] broadcast dim at position `dim` of ap.ap."""
    l = [list(d) for d in ap.ap]
    l.insert(dim, [0, count])
    return bass.AP(ap.tensor, ap.offset, l)


def build_nc(n_steps=T):
    nc = bacc.Bacc("TRN2", target_bir_lowering=False, debug=False, num_devices=NC)

    # ---- kernel I/O (per-core shards; all pre-laid-out on host) ----
    xet = nc.dram_tensor("xet", [H, T2], DT.bfloat16, kind="ExternalInput")
    xdt = nc.dram_tensor("xdt", [H, T2], DT.bfloat16, kind="ExternalInput")
    whe = nc.dram_tensor("whe", [128, KT * 512], DT.bfloat16, kind="ExternalInput")
    wie = nc.dram_tensor("wie", [128, KT * 512], DT.bfloat16, kind="ExternalInput")
    whd = nc.dram_tensor("whd", [128, KT * 512], DT.bfloat16, kind="ExternalInput")
    wid = nc.dram_tensor("wid", [128, KT * 512], DT.bfloat16, kind="ExternalInput")
    be = nc.dram_tensor("be", [128, 4], DT.float32, kind="ExternalInput")
    bd = nc.dram_tensor("bd", [128, 4], DT.float32, kind="ExternalInput")
    fw1 = nc.dram_tensor("fw1", [128, MT * KT * 128], DT.bfloat16, kind="ExternalInput")
    fw2 = nc.dram_tensor("fw2", [128, MT * KT * 128], DT.bfloat16, kind="ExternalInput")
    fcb = nc.dram_tensor("fcb", [128, MT], DT.float32, kind="ExternalInput")
    wet = nc.dram_tensor("wet", [128, KT], DT.bfloat16, kind="ExternalInput")
    out = nc.dram_tensor("out", [VLP, T2], DT.float32, kind="ExternalOutput")

    with tile.TileContext(nc) as tc:
        with (
            tc.tile_pool(name="persist", bufs=1) as pp,
            tc.tile_pool(name="work", bufs=2) as wk,
            tc.tile_pool(name="wstream", bufs=3) as ws,
            tc.tile_pool(name="dcc", bufs=4, space="DRAM") as dcc,
            tc.tile_pool(name="paux", bufs=3, space="PSUM") as paux,
            tc.tile_pool(name="pgate", bufs=1, space="PSUM") as pgate,
        ):
            # ---- persistent SBUF tensors ----
            whe_sb = pp.tile([128, KT * 512], DT.bfloat16, tag="whe")
            whd_sb = pp.tile([128, KT * 512], DT.bfloat16, tag="whd")
            wie_sb = pp.tile([128, KT * 512], DT.bfloat16, tag="wie")
            wid_sb = pp.tile([128, KT * 512], DT.bfloat16, tag="wid")
            be_sb = pp.tile([128, 4], DT.float32, tag="be")
            bd_sb = pp.tile([128, 4], DT.float32, tag="bd")
            fcb_sb = pp.tile([128, MT], DT.float32, tag="fcb")
            wet_sb = pp.tile([128, KT], DT.bfloat16, tag="wet")
            xt_sb = pp.tile([128, KT * T2], DT.bfloat16, tag="xt")       # 32KB/p
            xwe_sb = pp.tile([128, 4 * T2], DT.bfloat16, tag="xwe")      # 16KB/p
            xwd_sb = pp.tile([128, 4 * T2], DT.bfloat16, tag="xwd")      # 16KB/p
            hist_e = pp.tile([128, n_steps * 128], DT.bfloat16, tag="he")
            hist_d = pp.tile([128, n_steps * 128], DT.bfloat16, tag="hd")
            h0_sb = pp.tile([128, 128], DT.bfloat16, tag="h0")
            c_sb = pp.tile([128, B], DT.float32, tag="c")
            se_sb = pp.tile([1, T2], DT.float32, tag="se")
            seT_sb = pp.tile([B, n_steps], DT.float32, tag="seT")
            attn_sb = pp.tile([B, n_steps], DT.bfloat16, tag="attn")
            abc_sb = pp.tile([128, B * n_steps], DT.bfloat16, tag="abc")
            ctx_sb = pp.tile([128, KT * B], DT.bfloat16, tag="ctx")
            bias2_sb = pp.tile([128, MT * B], DT.float32, tag="bias2")

            nc.sync.dma_start(whe_sb[:], whe[:])
            nc.sync.dma_start(whd_sb[:], whd[:])
            nc.sync.dma_start(wie_sb[:], wie[:])
            nc.sync.dma_start(wid_sb[:], wid[:])
            nc.sync.dma_start(be_sb[:], be[:])
            nc.sync.dma_start(bd_sb[:], bd[:])
            nc.sync.dma_start(fcb_sb[:], fcb[:])
            nc.sync.dma_start(wet_sb[:], wet[:])
            nc.vector.memset(h0_sb[:], 0.0)
            nc.vector.memset(c_sb[:], 0.0)

            def load_xt(src):
                # [H, T2] dram -> [128, (k tau)] sbuf
                nc.sync.dma_start(
                    xt_sb[:].rearrange("p (k n) -> p k n", k=KT),
                    src[:].rearrange("(k p) n -> p k n", p=128),
                )

            def xw_chunk(xw_sb, wih_sb, b_sb, j, nb):
                """One (gate j, 512-token block nb) chunk of xw = x @ Wih_m.T + b."""
                ps = paux.tile([128, 512], DT.float32, tag="aux")
                for k in range(KT):
                    nc.tensor.matmul(
                        ps[:],
                        lhsT=wih_sb[:, k * 512 + j * 128 : k * 512 + (j + 1) * 128],
                        rhs=xt_sb[:, k * T2 + nb * 512 : k * T2 + (nb + 1) * 512],
                        start=(k == 0),
                        stop=(k == KT - 1),
                    )
                nc.vector.tensor_scalar_add(
                    xw_sb[:, j * T2 + nb * 512 : j * T2 + (nb + 1) * 512],
                    ps[:],
                    b_sb[:, j : j + 1],
                )

            def lstm_step(hist, hprev_ap, xw_sb, whh_sb, t):
                pg = pgate.tile([128, 2048], DT.float32, tag="g")
                for j in range(4):
                    for k in range(KT):
                        nc.tensor.matmul(
                            pg[:, j * 512 : j * 512 + B],
                            lhsT=whh_sb[:, k * 512 + j * 128 : k * 512 + (j + 1) * 128],
                            rhs=hprev_ap[:, k * B : (k + 1) * B],
                            start=(k == 0),
                            stop=(k == KT - 1),
                        )
                g_in = wk.tile([128, 64], DT.float32, tag="gin")
                nc.vector.tensor_add(
                    g_in[:].rearrange("p (j b) -> p j b", j=4),
                    pg[:].rearrange("p (j x) -> p j x", j=4)[:, :, 0:B],
                    xw_sb[:].rearrange("p (j n) -> p j n", j=4)[:, :, t * B : (t + 1) * B],
                )
                a = wk.tile([128, 64], DT.float32, tag="act")
                nc.scalar.activation(a[:, 0:48], g_in[:, 0:48], AF.Sigmoid)
                nc.scalar.activation(a[:, 48:64], g_in[:, 48:64], AF.Tanh)
                m1 = wk.tile([128, B], DT.float32, tag="m1")
                m2 = wk.tile([128, B], DT.float32, tag="m2")
                nc.vector.tensor_mul(m1[:], a[:, 0:16], c_sb[:])
                nc.vector.tensor_mul(m2[:], a[:, 16:32], a[:, 48:64])
                nc.vector.tensor_add(c_sb[:], m1[:], m2[:])
                tct = wk.tile([128, B], DT.float32, tag="tct")
                nc.scalar.activation(tct[:], c_sb[:], AF.Tanh)
                h_sb = wk.tile([128, B], DT.bfloat16, tag="h")
                nc.vector.tensor_mul(h_sb[:], a[:, 32:48], tct[:])
                # exchange: AllGather h slices -> full h in hist[:, t*128:(t+1)*128]
                cin = dcc.tile([128, B], DT.bfloat16, tag="cin")
                cout = dcc.tile([NC * 128, B], DT.bfloat16, tag="cout")
                nc.sync.dma_start(cin[:], h_sb[:])
                nc.gpsimd.collective_compute(
                    "AllGather",
                    ALU.bypass,
                    replica_groups=[list(range(NC))],
                    ins=[cin[:].opt()],
                    outs=[cout[:].opt()],
                )
                nc.sync.dma_start(
                    hist[:, t * 128 : (t + 1) * 128].rearrange("p (k b) -> p k b", k=KT),
                    cout[:].rearrange("(k p) b -> p k b", p=128),
                )

            # ================= background task machinery =================
            def fc_chunk(mj, nb):
                f1t = ws.tile([128, KT * 128], DT.bfloat16, tag="f1t")
                nc.sync.dma_start(f1t[:], fw1[:, mj * 1024 : (mj + 1) * 1024])
                ps = paux.tile([128, 512], DT.float32, tag="aux")
                hv = hist_d[:].rearrange("p (t k b) -> p t k b", t=n_steps, k=KT)
                for k in range(KT):
                    nc.tensor.matmul(
                        ps[:],
                        lhsT=f1t[:, k * 128 : (k + 1) * 128],
                        rhs=hv[:, nb * 32 : (nb + 1) * 32, k : k + 1, :],
                        start=(k == 0),
                        stop=(k == KT - 1),
                    )
                fco = ws.tile([128, 512], DT.float32, tag="fco")
                b2 = bias2_sb[:].rearrange("p (mj b) -> p mj b", mj=MT)[:, mj : mj + 1, :]
                nc.vector.tensor_tensor(
                    fco[:].rearrange("p (t b) -> p t b", t=32),
                    ps[:].rearrange("p (t b) -> p t b", t=32),
                    _bcast(b2, 1, 32),  # [128, 32(bcast), 1, 16]
                    op=ALU.add,
                )
                nc.sync.dma_start(
                    out[mj * 128 : (mj + 1) * 128, nb * 512 : (nb + 1) * 512], fco[:]
                )

            def se_quarter(q):
                ps = paux.tile([1, 512], DT.float32, tag="aux")
                hv = hist_e[:].rearrange("p (t k b) -> p t k b", t=n_steps, k=KT)
                for k in range(KT):
                    nc.tensor.matmul(
                        ps[:],
                        lhsT=wet_sb[:, k : k + 1],
                        rhs=hv[:, q * 32 : (q + 1) * 32, k : k + 1, :],
                        start=(k == 0),
                        stop=(k == KT - 1),
                    )
                nc.scalar.activation(se_sb[:, q * 512 : (q + 1) * 512], ps[:], AF.Identity)

            se_dram = dcc.tile([1, T2], DT.float32, tag="sed")
            abc_dram = dcc.tile([B, n_steps], DT.bfloat16, tag="abcd")

            def softmax_pieces():
                # emitted over several steps to hide DMA latency
                yield lambda: nc.sync.dma_start(se_dram[:], se_sb[:])
                # se layout: free = e*16 + b; load transposed [b, e]
                yield lambda: nc.sync.dma_start(
                    seT_sb[:],
                    bass.AP(se_dram[:].tensor, se_dram[:].offset, [[1, B], [B, n_steps]]),
                )

                def red():
                    mx = wk.tile([B, 1], DT.float32, tag="mx")
                    nc.vector.reduce_max(mx[:], seT_sb[:], axis=mybir.AxisListType.X)
                    nmx = wk.tile([B, 1], DT.float32, tag="nmx")
                    nc.vector.tensor_scalar(nmx[:], mx[:], -1.0, None, op0=ALU.mult)
                    ex = wk.tile([B, n_steps], DT.float32, tag="ex")
                    nc.scalar.activation(ex[:], seT_sb[:], AF.Exp, bias=nmx[:])
                    sm = wk.tile([B, 1], DT.float32, tag="sm")
                    nc.vector.reduce_sum(sm[:], ex[:], axis=mybir.AxisListType.X)
                    rs = wk.tile([B, 1], DT.float32, tag="rs")
                    nc.vector.reciprocal(rs[:], sm[:])
                    nc.vector.tensor_scalar(attn_sb[:], ex[:], rs[:], None, op0=ALU.mult)

                yield red
                yield lambda: nc.sync.dma_start(abc_dram[:], attn_sb[:])
                # broadcast [B*n_steps] to all 128 partitions
                yield lambda: nc.sync.dma_start(
                    abc_sb[:],
                    bass.AP(abc_dram[:].tensor, abc_dram[:].offset, [[0, 128], [1, B * n_steps]]),
                )

            def ctx_chunk(k, bh):
                # half bh of batches for h-tile k: ctx[k*16+b] = sum_e hist_e * attn
                tmp = wk.tile([128, 8 * n_steps], DT.float32, tag="ctmp")
                he = hist_e[:].rearrange("p (e k b) -> p k b e", e=n_steps, k=KT)
                ab = abc_sb[:].rearrange("p (b e) -> p b e", b=B)
                b0 = bh * 8
                nc.vector.tensor_tensor(
                    tmp[:].rearrange("p (b e) -> p b e", b=8),
                    he[:, k, b0 : b0 + 8, :],
                    ab[:, b0 : b0 + 8, :],
                    op=ALU.mult,
                )
                ctf = wk.tile([128, 8], DT.float32, tag="ctf")
                nc.vector.reduce_sum(
                    ctf[:],
                    tmp[:].rearrange("p (b e) -> p b e", b=8),
                    axis=mybir.AxisListType.X,
                )
                nc.vector.tensor_copy(ctx_sb[:, k * B + b0 : k * B + b0 + 8], ctf[:])

            def bias2_chunk(mj):
                f2t = ws.tile([128, KT * 128], DT.bfloat16, tag="f2t")
                nc.sync.dma_start(f2t[:], fw2[:, mj * 1024 : (mj + 1) * 1024])
                ps = paux.tile([128, B], DT.float32, tag="aux")
                for k in range(KT):
                    nc.tensor.matmul(
                        ps[:],
                        lhsT=f2t[:, k * 128 : (k + 1) * 128],
                        rhs=ctx_sb[:, k * B : (k + 1) * B],
                        start=(k == 0),
                        stop=(k == KT - 1),
                    )
                nc.scalar.activation(
                    bias2_sb[:, mj * B : (mj + 1) * B],
                    ps[:],
                    AF.Identity,
                    bias=fcb_sb[:, mj : mj + 1],
                )

            # ================= emission =================
            load_xt(xet)
            for j in range(4):
                for nb in range(4):
                    xw_chunk(xwe_sb, wie_sb, be_sb, j, nb)

            # encoder phase; background: load x_dec, compute xw_dec
            enc_bg = [lambda s=src: load_xt(s) for src in (xdt,)]
            enc_bg += [
                (lambda j=j, nb=nb: xw_chunk(xwd_sb, wid_sb, bd_sb, j, nb))
                for j in range(4)
                for nb in range(4)
            ]
            for t in range(n_steps):
                hprev = h0_sb[:] if t == 0 else hist_e[:, (t - 1) * 128 : t * 128]
                lstm_step(hist_e, hprev, xwe_sb, whe_sb, t)
                if t >= 2 and t % 6 == 2 and enc_bg:
                    enc_bg.pop(0)()

            # decoder phase; background: attention, bias2, then fc chunks
            dec_bg = [(lambda q=q: se_quarter(q)) for q in range(4)]
            dec_bg += list(softmax_pieces())
            dec_bg += [
                (lambda k=k, bh=bh: ctx_chunk(k, bh)) for k in range(KT) for bh in range(2)
            ]
            dec_bg += [(lambda mj=mj: bias2_chunk(mj)) for mj in range(MT)]
            fc_ready = {nb: 32 * (nb + 1) + 1 for nb in range(4)}
            fc_tasks = [(nb, mj) for nb in range(4) for mj in range(MT)]
            fc_i = 0
            for t in range(n_steps):
                hprev = (
                    hist_e[:, (n_steps - 1) * 128 : n_steps * 128]
                    if t == 0
                    else hist_d[:, (t - 1) * 128 : t * 128]
                )
                lstm_step(hist_d, hprev, xwd_sb, whd_sb, t)
                if t >= 1 and dec_bg:
                    dec_bg.pop(0)()
                    if dec_bg:
                        dec_bg.pop(0)()
                elif fc_i < len(fc_tasks) and t >= fc_ready[fc_tasks[fc_i][0]]:
                    nb, mj = fc_tasks[fc_i]
                    fc_chunk(mj, nb)
                    fc_i += 1
            while fc_i < len(fc_tasks):
                nb, mj = fc_tasks[fc_i]
                fc_chunk(mj, nb)
                fc_i += 1

    return nc


# ---------------- host side ----------------


def _gate_rows(m):
    return np.concatenate(
        [np.arange(g * H + m * HL, g * H + (m + 1) * HL) for g in GATE_SRC]
    )


def _pack_whh(w, rows):
    """[4H, H] weight -> per-core [128, KT*512] bf16 sbuf layout (k, j, c)."""
    lhsT = np.ascontiguousarray(w[rows].T)  # [1024, 512]
    return (
        lhsT.reshape(KT, 128, 4, 128).transpose(1, 0, 2, 3).reshape(128, KT * 512)
    ).astype(BF16)


def _pack_fc(wpart):
    """[4096, 1024] -> [128, MT*KT*128] bf16 layout (mj, k, c)."""
    lhsT = np.ascontiguousarray(wpart.T)  # [1024, 4096]
    return (
        lhsT.reshape(KT, 128, MT, 128).transpose(1, 2, 0, 3).reshape(128, MT * KT * 128)
    ).astype(BF16)


def _xT(emb_rows):
    """[B, T, H] f32 -> [H, T2] bf16 with tau = t*B + b."""
    xt = np.transpose(emb_rows, (1, 0, 2)).reshape(T2, H)
    return np.ascontiguousarray(xt.T).astype(BF16)


_NC_CACHE = {}


def _get_nc():
    if "nc" not in _NC_CACHE:
        _NC_CACHE["nc"] = build_nc()
    return _NC_CACHE["nc"]


def make_in_maps(
    src, tgt, src_emb, tgt_emb, enc_Wih, enc_Whh, enc_bih, enc_bhh,
    dec_Wih, dec_Whh, dec_bih, dec_bhh, attn_w, attn_b, fc_w, fc_b,
):
    src = np.asarray(src)
    tgt = np.asarray(tgt)
    xet = _xT(np.asarray(src_emb, np.float32)[src])
    xdt = _xT(np.asarray(tgt_emb, np.float32)[tgt])
    b_enc = np.asarray(enc_bih, np.float32) + np.asarray(enc_bhh, np.float32)
    b_dec = np.asarray(dec_bih, np.float32) + np.asarray(dec_bhh, np.float32)
    we = np.asarray(attn_w, np.float32)[0, H:]
    wet = np.ascontiguousarray(we.reshape(KT, 128).T).astype(BF16)
    fc_w = np.asarray(fc_w, np.float32)
    fc_b = np.asarray(fc_b, np.float32)

    in_maps = []
    for m in range(NC):
        rows = _gate_rows(m)
        vlo = m * VL
        wrows = np.zeros((VLP, 2 * H), np.float32)
        nreal = min(VLP, V - vlo)
        wrows[:nreal] = fc_w[vlo : vlo + nreal]
        brows = np.zeros((VLP,), np.float32)
        brows[:nreal] = fc_b[vlo : vlo + nreal]
        in_maps.append(
            {
                "xet": xet,
                "xdt": xdt,
                "whe": _pack_whh(np.asarray(enc_Whh, np.float32), rows),
                "wie": _pack_whh(np.asarray(enc_Wih, np.float32), rows),
                "whd": _pack_whh(np.asarray(dec_Whh, np.float32), rows),
                "wid": _pack_whh(np.asarray(dec_Wih, np.float32), rows),
                "be": np.ascontiguousarray(b_enc[rows].reshape(4, 128).T),
                "bd": np.ascontiguousarray(b_dec[rows].reshape(4, 128).T),
                "fw1": _pack_fc(wrows[:, :H]),
                "fw2": _pack_fc(wrows[:, H:]),
                "fcb": np.ascontiguousarray(brows.reshape(MT, 128).T),
                "wet": wet,
            }
        )
    return in_maps


def kernel(**inputs):
    nc = _get_nc()
    in_maps = make_in_maps(**inputs)
    res = run_bass_kernel_spmd(nc, in_maps, core_ids=list(range(NC)))
    shards = [np.asarray(r["out"], np.float32)[:VL] for r in res.results]
    full = np.concatenate(shards, axis=0)  # [V, T2]
    return np.ascontiguousarray(full.reshape(V, T, B).transpose(2, 1, 0))


# revision 6
# speedup vs baseline: 1.2957x; 1.2957x over previous
"""Trainium2 8-core kernel for the LSTM seq2seq + attention + vocab-projection model.

Strategy:
  - LSTM recurrence: tensor-parallel over the gate dimension. Core m owns
    h-slice [m*128:(m+1)*128) and computes the 4 gate rows for that slice
    (packed on host in order [f, i, o, g], 128 rows each). After each step the
    h-slices are AllGathered (bf16, 8KB) so every core holds the full h for
    the next step's matmul. Cell state c stays sharded; it is never exchanged.
  - The additive attention collapses: softmax over (dec@wd + enc@we + b) along
    the encoder axis is independent of the decoder position, so the context
    vector is per-batch constant. ctx[b] = softmax_e(enc_out[b]@we) @ enc_out[b].
  - Final projection is vocab-sharded: core m computes rows [m*4000,(m+1)*4000)
    (padded to 4096) of  out = dec_out @ fc_w[:, :H].T + (ctx @ fc_w[:, H:].T
    + fc_b).  The fc work is emitted as background chunks interleaved into the
    decoder phase so it runs in PE gaps while steps wait on the AllGather.
Token index convention: tau = t*16 + b  (time-major, batch inner).
"""

import os
import sys

for _p in ("/opt/trn_rl_repo", "/root/.axon_site/_ro/trn_rl_repo"):
    if os.path.isdir(_p) and _p not in sys.path:
        sys.path.insert(0, _p)

import numpy as np
import ml_dtypes

import concourse.bass as bass
import concourse.bacc as bacc
import concourse.tile as tile
from concourse import mybir
from concourse.bass_utils import run_bass_kernel_spmd

BF16 = ml_dtypes.bfloat16
DT = mybir.dt
AF = mybir.ActivationFunctionType
ALU = mybir.AluOpType

B = 16
T = 128          # both encoder and decoder length
H = 1024
V = 32000
NC = 8
HL = H // NC     # 128  h-slice per core
KT = H // 128    # 8    K tiles of the hidden dim
T2 = B * T       # 2048 tokens
VL = V // NC     # 4000 real vocab rows per core
VLP = 4096       # padded vocab rows per core
MT = VLP // 128  # 32   vocab M-tiles per core
# gate order on device: [f, i, o, g]; torch rows are [i, f, g, o]
GATE_SRC = (1, 0, 3, 2)


def _bcast(ap, dim, count):
    """Insert a [step=0, count] broadcast dim at position `dim` of ap.ap."""
    l = [list(d) for d in ap.ap]
    l.insert(dim, [0, count])
    return bass.AP(ap.tensor, ap.offset, l)


def build_nc(n_steps=T, reps=1):
    nc = bacc.Bacc("TRN2", target_bir_lowering=False, debug=False, num_devices=NC)

    # ---- kernel I/O (per-core shards; all pre-laid-out on host) ----
    xet = nc.dram_tensor("xet", [H, T2], DT.bfloat16, kind="ExternalInput")
    xdt = nc.dram_tensor("xdt", [H, T2], DT.bfloat16, kind="ExternalInput")
    whe = nc.dram_tensor("whe", [128, KT * 512], DT.bfloat16, kind="ExternalInput")
    wie = nc.dram_tensor("wie", [128, KT * 512], DT.bfloat16, kind="ExternalInput")
    whd = nc.dram_tensor("whd", [128, KT * 512], DT.bfloat16, kind="ExternalInput")
    wid = nc.dram_tensor("wid", [128, KT * 512], DT.bfloat16, kind="ExternalInput")
    be = nc.dram_tensor("be", [128, 4], DT.float32, kind="ExternalInput")
    bd = nc.dram_tensor("bd", [128, 4], DT.float32, kind="ExternalInput")
    fw1 = nc.dram_tensor("fw1", [128, MT * KT * 128], DT.bfloat16, kind="ExternalInput")
    fw2 = nc.dram_tensor("fw2", [128, MT * KT * 128], DT.bfloat16, kind="ExternalInput")
    fcb = nc.dram_tensor("fcb", [128, MT], DT.float32, kind="ExternalInput")
    wet = nc.dram_tensor("wet", [128, KT], DT.bfloat16, kind="ExternalInput")
    out = nc.dram_tensor("out", [VLP, T2], DT.float32, kind="ExternalOutput")

    with tile.TileContext(nc) as tc:
        with (
            tc.tile_pool(name="persist", bufs=1) as pp,
            tc.tile_pool(name="work", bufs=2) as wk,
            tc.tile_pool(name="wstream", bufs=3) as ws,
            tc.tile_pool(name="dcc", bufs=4, space="DRAM") as dcc,
            tc.tile_pool(name="paux", bufs=3, space="PSUM") as paux,
            tc.tile_pool(name="pgate", bufs=1, space="PSUM") as pgate,
        ):
            for _rep in range(reps):
                _emit_once(
                    nc, tc, pp, wk, ws, dcc, paux, pgate, n_steps,
                    xet, xdt, whe, wie, whd, wid, be, bd, fw1, fw2, fcb, wet, out,
                )

    nc.compile()
    return nc


def _emit_once(
    nc, tc, pp, wk, ws, dcc, paux, pgate, n_steps,
    xet, xdt, whe, wie, whd, wid, be, bd, fw1, fw2, fcb, wet, out,
):
    if True:
        if True:
            # ---- persistent SBUF tensors ----
            whe_sb = pp.tile([128, KT * 512], DT.bfloat16, tag="whe")
            whd_sb = pp.tile([128, KT * 512], DT.bfloat16, tag="whd")
            wie_sb = pp.tile([128, KT * 512], DT.bfloat16, tag="wie")
            wid_sb = pp.tile([128, KT * 512], DT.bfloat16, tag="wid")
            be_sb = pp.tile([128, 4], DT.float32, tag="be")
            bd_sb = pp.tile([128, 4], DT.float32, tag="bd")
            fcb_sb = pp.tile([128, MT], DT.float32, tag="fcb")
            wet_sb = pp.tile([128, KT], DT.bfloat16, tag="wet")
            xt_sb = pp.tile([128, KT * T2], DT.bfloat16, tag="xt")       # 32KB/p
            xwe_sb = pp.tile([128, 4 * T2], DT.bfloat16, tag="xwe")      # 16KB/p
            xwd_sb = pp.tile([128, 4 * T2], DT.bfloat16, tag="xwd")      # 16KB/p
            hist_e = pp.tile([128, n_steps * 128], DT.bfloat16, tag="he")
            hist_d = pp.tile([128, n_steps * 128], DT.bfloat16, tag="hd")
            h0_sb = pp.tile([128, 128], DT.bfloat16, tag="h0")
            c_sb = pp.tile([128, B], DT.float32, tag="c")
            se_sb = pp.tile([1, T2], DT.float32, tag="se")
            seT_sb = pp.tile([B, n_steps], DT.float32, tag="seT")
            attn_sb = pp.tile([B, n_steps], DT.bfloat16, tag="attn")
            abc_sb = pp.tile([128, B * n_steps], DT.bfloat16, tag="abc")
            ctx_sb = pp.tile([128, KT * B], DT.bfloat16, tag="ctx")
            bias2_sb = pp.tile([128, MT * B], DT.float32, tag="bias2")

            nc.sync.dma_start(whe_sb[:], whe[:])
            nc.sync.dma_start(whd_sb[:], whd[:])
            nc.sync.dma_start(wie_sb[:], wie[:])
            nc.sync.dma_start(wid_sb[:], wid[:])
            nc.sync.dma_start(be_sb[:], be[:])
            nc.sync.dma_start(bd_sb[:], bd[:])
            nc.sync.dma_start(fcb_sb[:], fcb[:])
            nc.sync.dma_start(wet_sb[:], wet[:])
            nc.vector.memset(h0_sb[:], 0.0)
            nc.vector.memset(c_sb[:], 0.0)

            def load_xt(src):
                # [H, T2] dram -> [128, (k tau)] sbuf
                nc.sync.dma_start(
                    xt_sb[:].rearrange("p (k n) -> p k n", k=KT),
                    src[:].rearrange("(k p) n -> p k n", p=128),
                )

            def xw_chunk(xw_sb, wih_sb, b_sb, j, nb):
                """One (gate j, 512-token block nb) chunk of xw = x @ Wih_m.T + b."""
                ps = paux.tile([128, 512], DT.float32, tag="aux")
                for k in range(KT):
                    nc.tensor.matmul(
                        ps[:],
                        lhsT=wih_sb[:, k * 512 + j * 128 : k * 512 + (j + 1) * 128],
                        rhs=xt_sb[:, k * T2 + nb * 512 : k * T2 + (nb + 1) * 512],
                        start=(k == 0),
                        stop=(k == KT - 1),
                    )
                nc.vector.tensor_scalar_add(
                    xw_sb[:, j * T2 + nb * 512 : j * T2 + (nb + 1) * 512],
                    ps[:],
                    b_sb[:, j : j + 1],
                )

            def lstm_step(hist, hprev_ap, xw_sb, whh_sb, t):
                pg = pgate.tile([128, 2048], DT.float32, tag="g")
                for j in range(4):
                    for k in range(KT):
                        nc.tensor.matmul(
                            pg[:, j * 512 : j * 512 + B],
                            lhsT=whh_sb[:, k * 512 + j * 128 : k * 512 + (j + 1) * 128],
                            rhs=hprev_ap[:, k * B : (k + 1) * B],
                            start=(k == 0),
                            stop=(k == KT - 1),
                        )
                g_in = wk.tile([128, 64], DT.float32, tag="gin")
                nc.vector.tensor_add(
                    g_in[:].rearrange("p (j b) -> p j b", j=4),
                    pg[:].rearrange("p (j x) -> p j x", j=4)[:, :, 0:B],
                    xw_sb[:].rearrange("p (j n) -> p j n", j=4)[:, :, t * B : (t + 1) * B],
                )
                a = wk.tile([128, 64], DT.float32, tag="act")
                nc.scalar.activation(a[:, 0:48], g_in[:, 0:48], AF.Sigmoid)
                nc.scalar.activation(a[:, 48:64], g_in[:, 48:64], AF.Tanh)
                m1 = wk.tile([128, B], DT.float32, tag="m1")
                m2 = wk.tile([128, B], DT.float32, tag="m2")
                nc.vector.tensor_mul(m1[:], a[:, 0:16], c_sb[:])
                nc.vector.tensor_mul(m2[:], a[:, 16:32], a[:, 48:64])
                nc.vector.tensor_add(c_sb[:], m1[:], m2[:])
                tct = wk.tile([128, B], DT.float32, tag="tct")
                nc.scalar.activation(tct[:], c_sb[:], AF.Tanh)
                h_sb = wk.tile([128, B], DT.bfloat16, tag="h")
                nc.vector.tensor_mul(h_sb[:], a[:, 32:48], tct[:])
                # exchange: AllGather h slices -> full h in hist[:, t*128:(t+1)*128]
                cin = dcc.tile([128, B], DT.bfloat16, tag="cin")
                cout = dcc.tile([NC * 128, B], DT.bfloat16, tag="cout")
                nc.sync.dma_start(cin[:], h_sb[:])
                nc.gpsimd.collective_compute(
                    "AllGather",
                    ALU.bypass,
                    replica_groups=[list(range(NC))],
                    ins=[cin[:].opt()],
                    outs=[cout[:].opt()],
                )
                nc.sync.dma_start(
                    hist[:, t * 128 : (t + 1) * 128].rearrange("p (k b) -> p k b", k=KT),
                    cout[:].rearrange("(k p) b -> p k b", p=128),
                )

            # ================= background task machinery =================
            def fc_chunk(mj, nb):
                f1t = ws.tile([128, KT * 128], DT.bfloat16, tag="f1t")
                nc.sync.dma_start(f1t[:], fw1[:, mj * 1024 : (mj + 1) * 1024])
                ps = paux.tile([128, 512], DT.float32, tag="aux")
                hv = hist_d[:].rearrange("p (t k b) -> p t k b", t=n_steps, k=KT)
                for k in range(KT):
                    nc.tensor.matmul(
                        ps[:],
                        lhsT=f1t[:, k * 128 : (k + 1) * 128],
                        rhs=hv[:, nb * 32 : (nb + 1) * 32, k : k + 1, :],
                        start=(k == 0),
                        stop=(k == KT - 1),
                    )
                fco = ws.tile([128, 512], DT.float32, tag="fco")
                b2 = bias2_sb[:, mj * B : (mj + 1) * B]  # [128, 16]
                nc.vector.tensor_tensor(
                    fco[:].rearrange("p (t b) -> p t b", t=32),
                    ps[:].rearrange("p (t b) -> p t b", t=32),
                    _bcast(b2, 1, 32),  # [128, 32(bcast), 16]
                    op=ALU.add,
                )
                nc.sync.dma_start(
                    out[mj * 128 : (mj + 1) * 128, nb * 512 : (nb + 1) * 512], fco[:]
                )

            def se_quarter(q):
                ps = paux.tile([1, 512], DT.float32, tag="aux")
                hv = hist_e[:].rearrange("p (t k b) -> p t k b", t=n_steps, k=KT)
                for k in range(KT):
                    nc.tensor.matmul(
                        ps[:],
                        lhsT=wet_sb[:, k : k + 1],
                        rhs=hv[:, q * 32 : (q + 1) * 32, k : k + 1, :],
                        start=(k == 0),
                        stop=(k == KT - 1),
                    )
                nc.scalar.activation(se_sb[:, q * 512 : (q + 1) * 512], ps[:], AF.Identity)

            se_dram = dcc.tile([1, T2], DT.float32, tag="sed")
            abc_dram = dcc.tile([B, n_steps], DT.bfloat16, tag="abcd")

            def softmax_pieces():
                # emitted over several steps to hide DMA latency
                yield lambda: nc.sync.dma_start(se_dram[:], se_sb[:])
                # se layout: free = e*16 + b; load transposed [b, e]
                yield lambda: nc.sync.dma_start(
                    seT_sb[:],
                    bass.AP(se_dram[:].tensor, se_dram[:].offset, [[1, B], [B, n_steps]]),
                )

                def red():
                    mx = wk.tile([B, 1], DT.float32, tag="mx")
                    nc.vector.reduce_max(mx[:], seT_sb[:], axis=mybir.AxisListType.X)
                    nmx = wk.tile([B, 1], DT.float32, tag="nmx")
                    nc.vector.tensor_scalar(nmx[:], mx[:], -1.0, None, op0=ALU.mult)
                    ex = wk.tile([B, n_steps], DT.float32, tag="ex")
                    nc.scalar.activation(ex[:], seT_sb[:], AF.Exp, bias=nmx[:])
                    sm = wk.tile([B, 1], DT.float32, tag="sm")
                    nc.vector.reduce_sum(sm[:], ex[:], axis=mybir.AxisListType.X)
                    rs = wk.tile([B, 1], DT.float32, tag="rs")
                    nc.vector.reciprocal(rs[:], sm[:])
                    nc.vector.tensor_scalar(attn_sb[:], ex[:], rs[:], None, op0=ALU.mult)

                yield red
                yield lambda: nc.sync.dma_start(abc_dram[:], attn_sb[:])
                # broadcast [B*n_steps] to all 128 partitions
                yield lambda: nc.sync.dma_start(
                    abc_sb[:],
                    bass.AP(abc_dram[:].tensor, abc_dram[:].offset, [[0, 128], [1, B * n_steps]]),
                )

            def ctx_chunk(k, bh):
                # half bh of batches for h-tile k: ctx[k*16+b] = sum_e hist_e * attn
                tmp = wk.tile([128, 8 * n_steps], DT.float32, tag="ctmp")
                he = hist_e[:].rearrange("p (e k b) -> p k b e", e=n_steps, k=KT)
                ab = abc_sb[:].rearrange("p (b e) -> p b e", b=B)
                b0 = bh * 8
                nc.vector.tensor_tensor(
                    tmp[:].rearrange("p (b e) -> p b e", b=8),
                    he[:, k, b0 : b0 + 8, :],
                    ab[:, b0 : b0 + 8, :],
                    op=ALU.mult,
                )
                ctf = wk.tile([128, 8], DT.float32, tag="ctf")
                nc.vector.reduce_sum(
                    ctf[:],
                    tmp[:].rearrange("p (b e) -> p b e", b=8),
                    axis=mybir.AxisListType.X,
                )
                nc.vector.tensor_copy(ctx_sb[:, k * B + b0 : k * B + b0 + 8], ctf[:])

            def bias2_chunk(mj):
                f2t = ws.tile([128, KT * 128], DT.bfloat16, tag="f2t")
                nc.sync.dma_start(f2t[:], fw2[:, mj * 1024 : (mj + 1) * 1024])
                ps = paux.tile([128, B], DT.float32, tag="aux")
                for k in range(KT):
                    nc.tensor.matmul(
                        ps[:],
                        lhsT=f2t[:, k * 128 : (k + 1) * 128],
                        rhs=ctx_sb[:, k * B : (k + 1) * B],
                        start=(k == 0),
                        stop=(k == KT - 1),
                    )
                nc.scalar.activation(
                    bias2_sb[:, mj * B : (mj + 1) * B],
                    ps[:],
                    AF.Identity,
                    bias=fcb_sb[:, mj : mj + 1],
                )

            # ================= emission =================
            load_xt(xet)
            for j in range(4):
                for nb in range(4):
                    xw_chunk(xwe_sb, wie_sb, be_sb, j, nb)

            # encoder phase; background: load x_dec, compute xw_dec
            enc_bg = [lambda s=src: load_xt(s) for src in (xdt,)]
            enc_bg += [
                (lambda j=j, nb=nb: xw_chunk(xwd_sb, wid_sb, bd_sb, j, nb))
                for j in range(4)
                for nb in range(4)
            ]
            for t in range(n_steps):
                hprev = h0_sb[:] if t == 0 else hist_e[:, (t - 1) * 128 : t * 128]
                lstm_step(hist_e, hprev, xwe_sb, whe_sb, t)
                if t >= 2 and t % 6 == 2 and enc_bg:
                    enc_bg.pop(0)()

            # decoder phase; background: attention, bias2, then fc chunks
            dec_bg = [(lambda q=q: se_quarter(q)) for q in range(4)]
            dec_bg += list(softmax_pieces())
            dec_bg += [
                (lambda k=k, bh=bh: ctx_chunk(k, bh)) for k in range(KT) for bh in range(2)
            ]
            dec_bg += [(lambda mj=mj: bias2_chunk(mj)) for mj in range(MT)]
            fc_ready = {nb: 32 * (nb + 1) + 1 for nb in range(4)}
            fc_tasks = [(nb, mj) for nb in range(4) for mj in range(MT)]
            fc_i = 0
            for t in range(n_steps):
                hprev = (
                    hist_e[:, (n_steps - 1) * 128 : n_steps * 128]
                    if t == 0
                    else hist_d[:, (t - 1) * 128 : t * 128]
                )
                lstm_step(hist_d, hprev, xwd_sb, whd_sb, t)
                if t >= 1 and dec_bg:
                    dec_bg.pop(0)()
                    if dec_bg:
                        dec_bg.pop(0)()
                elif fc_i < len(fc_tasks) and t >= fc_ready[fc_tasks[fc_i][0]]:
                    nb, mj = fc_tasks[fc_i]
                    fc_chunk(mj, nb)
                    fc_i += 1
            while fc_i < len(fc_tasks):
                nb, mj = fc_tasks[fc_i]
                fc_chunk(mj, nb)
                fc_i += 1


# ---------------- host side ----------------


def _gate_rows(m):
    return np.concatenate(
        [np.arange(g * H + m * HL, g * H + (m + 1) * HL) for g in GATE_SRC]
    )


def _pack_whh(w, rows):
    """[4H, H] weight -> per-core [128, KT*512] bf16 sbuf layout (k, j, c)."""
    lhsT = np.ascontiguousarray(w[rows].T)  # [1024, 512]
    return (
        lhsT.reshape(KT, 128, 4, 128).transpose(1, 0, 2, 3).reshape(128, KT * 512)
    ).astype(BF16)


def _pack_fc(wpart):
    """[4096, 1024] -> [128, MT*KT*128] bf16 layout (mj, k, c)."""
    lhsT = np.ascontiguousarray(wpart.T)  # [1024, 4096]
    return (
        lhsT.reshape(KT, 128, MT, 128).transpose(1, 2, 0, 3).reshape(128, MT * KT * 128)
    ).astype(BF16)


def _xT(emb_rows):
    """[B, T, H] f32 -> [H, T2] bf16 with tau = t*B + b."""
    xt = np.transpose(emb_rows, (1, 0, 2)).reshape(T2, H)
    return np.ascontiguousarray(xt.T).astype(BF16)


_NC_CACHE = {}


def _get_nc():
    if "nc" not in _NC_CACHE:
        _NC_CACHE["nc"] = build_nc()
    return _NC_CACHE["nc"]


def make_in_maps(
    src, tgt, src_emb, tgt_emb, enc_Wih, enc_Whh, enc_bih, enc_bhh,
    dec_Wih, dec_Whh, dec_bih, dec_bhh, attn_w, attn_b, fc_w, fc_b,
):
    src = np.asarray(src)
    tgt = np.asarray(tgt)
    xet = _xT(np.asarray(src_emb, np.float32)[src])
    xdt = _xT(np.asarray(tgt_emb, np.float32)[tgt])
    b_enc = np.asarray(enc_bih, np.float32) + np.asarray(enc_bhh, np.float32)
    b_dec = np.asarray(dec_bih, np.float32) + np.asarray(dec_bhh, np.float32)
    we = np.asarray(attn_w, np.float32)[0, H:]
    wet = np.ascontiguousarray(we.reshape(KT, 128).T).astype(BF16)
    fc_w = np.asarray(fc_w, np.float32)
    fc_b = np.asarray(fc_b, np.float32)

    in_maps = []
    for m in range(NC):
        rows = _gate_rows(m)
        vlo = m * VL
        wrows = np.zeros((VLP, 2 * H), np.float32)
        nreal = min(VLP, V - vlo)
        wrows[:nreal] = fc_w[vlo : vlo + nreal]
        brows = np.zeros((VLP,), np.float32)
        brows[:nreal] = fc_b[vlo : vlo + nreal]
        in_maps.append(
            {
                "xet": xet,
                "xdt": xdt,
                "whe": _pack_whh(np.asarray(enc_Whh, np.float32), rows),
                "wie": _pack_whh(np.asarray(enc_Wih, np.float32), rows),
                "whd": _pack_whh(np.asarray(dec_Whh, np.float32), rows),
                "wid": _pack_whh(np.asarray(dec_Wih, np.float32), rows),
                "be": np.ascontiguousarray(b_enc[rows].reshape(4, 128).T),
                "bd": np.ascontiguousarray(b_dec[rows].reshape(4, 128).T),
                "fw1": _pack_fc(wrows[:, :H]),
                "fw2": _pack_fc(wrows[:, H:]),
                "fcb": np.ascontiguousarray(brows.reshape(MT, 128).T),
                "wet": wet,
            }
        )
    return in_maps


def kernel(**inputs):
    nc = _get_nc()
    in_maps = make_in_maps(**inputs)
    res = run_bass_kernel_spmd(nc, in_maps, core_ids=list(range(NC)))
    shards = [np.asarray(r["out"], np.float32)[:VL] for r in res.results]
    full = np.concatenate(shards, axis=0)  # [V, T2]
    return np.ascontiguousarray(full.reshape(V, T, B).transpose(2, 1, 0))


# revision 8
# speedup vs baseline: 1.8144x; 1.4003x over previous
"""Trainium2 8-core kernel for the LSTM seq2seq + attention + vocab-projection model.

Strategy:
  - LSTM recurrence: tensor-parallel over the gate dimension. Core m owns
    h-slice [m*128:(m+1)*128) and computes the 4 gate rows for that slice
    (packed on host in order [f, i, o, g], 128 rows each). After each step the
    h-slices are AllGathered (bf16, 8KB) so every core holds the full h for
    the next step's matmul. Cell state c stays sharded; it is never exchanged.
  - The additive attention collapses: softmax over (dec@wd + enc@we + b) along
    the encoder axis is independent of the decoder position, so the context
    vector is per-batch constant. ctx[b] = softmax_e(enc_out[b]@we) @ enc_out[b].
  - Final projection is vocab-sharded: core m computes rows [m*4000,(m+1)*4000)
    (padded to 4096) of  out = dec_out @ fc_w[:, :H].T + (ctx @ fc_w[:, H:].T
    + fc_b).  The fc work is emitted as background chunks interleaved into the
    decoder phase so it runs in PE gaps while steps wait on the AllGather.
Token index convention: tau = t*16 + b  (time-major, batch inner).
"""

import os
import sys

for _p in ("/opt/trn_rl_repo", "/root/.axon_site/_ro/trn_rl_repo"):
    if os.path.isdir(_p) and _p not in sys.path:
        sys.path.insert(0, _p)

import numpy as np
import ml_dtypes

import concourse.bass as bass
import concourse.bacc as bacc
import concourse.tile as tile
from concourse import mybir
from concourse.bass_utils import run_bass_kernel_spmd

BF16 = ml_dtypes.bfloat16
DT = mybir.dt
AF = mybir.ActivationFunctionType
ALU = mybir.AluOpType

B = 16
T = 128          # both encoder and decoder length
H = 1024
V = 32000
NC = 8
HL = H // NC     # 128  h-slice per core
KT = H // 128    # 8    K tiles of the hidden dim
T2 = B * T       # 2048 tokens
VL = V // NC     # 4000 real vocab rows per core
VLP = 4096       # padded vocab rows per core
MT = VLP // 128  # 32   vocab M-tiles per core
# gate order on device: [f, i, o, g]; torch rows are [i, f, g, o]
GATE_SRC = (1, 0, 3, 2)


def _bcast(ap, dim, count):
    """Insert a [step=0, count] broadcast dim at position `dim` of ap.ap."""
    l = [list(d) for d in ap.ap]
    l.insert(dim, [0, count])
    return bass.AP(ap.tensor, ap.offset, l)


CC_MODE = "cc"  # "cc" real AllGather | "dram" fake via dram copy | "sbuf" fake via sbuf copy


def build_nc(n_steps=T, reps=1):
    nc = bacc.Bacc("TRN2", target_bir_lowering=False, debug=False, num_devices=NC)

    # ---- kernel I/O (per-core shards; all pre-laid-out on host) ----
    xet = nc.dram_tensor("xet", [H, T2], DT.bfloat16, kind="ExternalInput")
    xdt = nc.dram_tensor("xdt", [H, T2], DT.bfloat16, kind="ExternalInput")
    whe = nc.dram_tensor("whe", [128, KT * 512], DT.bfloat16, kind="ExternalInput")
    wie = nc.dram_tensor("wie", [128, KT * 512], DT.bfloat16, kind="ExternalInput")
    whd = nc.dram_tensor("whd", [128, KT * 512], DT.bfloat16, kind="ExternalInput")
    wid = nc.dram_tensor("wid", [128, KT * 512], DT.bfloat16, kind="ExternalInput")
    be = nc.dram_tensor("be", [128, 4], DT.float32, kind="ExternalInput")
    bd = nc.dram_tensor("bd", [128, 4], DT.float32, kind="ExternalInput")
    fw1 = nc.dram_tensor("fw1", [128, MT * KT * 128], DT.bfloat16, kind="ExternalInput")
    fw2 = nc.dram_tensor("fw2", [128, MT * KT * 128], DT.bfloat16, kind="ExternalInput")
    fcb = nc.dram_tensor("fcb", [128, MT], DT.float32, kind="ExternalInput")
    wet = nc.dram_tensor("wet", [128, KT], DT.bfloat16, kind="ExternalInput")
    out = nc.dram_tensor("out", [VLP, T2], DT.float32, kind="ExternalOutput")

    with tile.TileContext(nc) as tc:
        with (
            tc.tile_pool(name="persist", bufs=1) as pp,
            tc.tile_pool(name="work", bufs=2) as wk,
            tc.tile_pool(name="wstream", bufs=3) as ws,
            tc.tile_pool(name="dcc", bufs=4, space="DRAM") as dcc,
            tc.tile_pool(name="paux", bufs=3, space="PSUM") as paux,
            tc.tile_pool(name="pgate", bufs=1, space="PSUM") as pgate,
        ):
            for _rep in range(reps):
                _emit_once(
                    nc, tc, pp, wk, ws, dcc, paux, pgate, n_steps,
                    xet, xdt, whe, wie, whd, wid, be, bd, fw1, fw2, fcb, wet, out,
                )

    nc.compile()
    return nc


def _emit_once(
    nc, tc, pp, wk, ws, dcc, paux, pgate, n_steps,
    xet, xdt, whe, wie, whd, wid, be, bd, fw1, fw2, fcb, wet, out,
):
    if True:
        if True:
            # ---- persistent SBUF tensors ----
            whe_sb = pp.tile([128, KT * 512], DT.bfloat16, tag="whe")
            whd_sb = pp.tile([128, KT * 512], DT.bfloat16, tag="whd")
            wie_sb = pp.tile([128, KT * 512], DT.bfloat16, tag="wie")
            wid_sb = pp.tile([128, KT * 512], DT.bfloat16, tag="wid")
            be_sb = pp.tile([128, 4], DT.float32, tag="be")
            bd_sb = pp.tile([128, 4], DT.float32, tag="bd")
            fcb_sb = pp.tile([128, MT], DT.float32, tag="fcb")
            wet_sb = pp.tile([128, KT], DT.bfloat16, tag="wet")
            xt_sb = pp.tile([128, KT * T2], DT.bfloat16, tag="xt")       # 32KB/p
            xwe_sb = pp.tile([128, 4 * T2], DT.bfloat16, tag="xwe")      # 16KB/p
            xwd_sb = pp.tile([128, 4 * T2], DT.bfloat16, tag="xwd")      # 16KB/p
            hist_e = pp.tile([128, n_steps * 128], DT.bfloat16, tag="he")
            hist_d = pp.tile([128, n_steps * 128], DT.bfloat16, tag="hd")
            h0_sb = pp.tile([128, 128], DT.bfloat16, tag="h0")
            c_sb = pp.tile([128, B], DT.float32, tag="c")
            se_sb = pp.tile([1, T2], DT.float32, tag="se")
            seT_sb = pp.tile([B, n_steps], DT.float32, tag="seT")
            attn_sb = pp.tile([B, n_steps], DT.bfloat16, tag="attn")
            abc_sb = pp.tile([128, B * n_steps], DT.bfloat16, tag="abc")
            ctx_sb = pp.tile([128, KT * B], DT.bfloat16, tag="ctx")
            bias2_sb = pp.tile([128, MT * B], DT.float32, tag="bias2")

            nc.sync.dma_start(whe_sb[:], whe[:])
            nc.sync.dma_start(whd_sb[:], whd[:])
            nc.sync.dma_start(wie_sb[:], wie[:])
            nc.sync.dma_start(wid_sb[:], wid[:])
            nc.sync.dma_start(be_sb[:], be[:])
            nc.sync.dma_start(bd_sb[:], bd[:])
            nc.sync.dma_start(fcb_sb[:], fcb[:])
            nc.sync.dma_start(wet_sb[:], wet[:])
            nc.vector.memset(h0_sb[:], 0.0)
            nc.vector.memset(c_sb[:], 0.0)

            def load_xt(src):
                # [H, T2] dram -> [128, (k tau)] sbuf
                nc.sync.dma_start(
                    xt_sb[:].rearrange("p (k n) -> p k n", k=KT),
                    src[:].rearrange("(k p) n -> p k n", p=128),
                )

            def xw_chunk(xw_sb, wih_sb, b_sb, j, nb):
                """One (gate j, 512-token block nb) chunk of xw = x @ Wih_m.T + b."""
                ps = paux.tile([128, 512], DT.float32, tag="aux")
                for k in range(KT):
                    nc.tensor.matmul(
                        ps[:],
                        lhsT=wih_sb[:, k * 512 + j * 128 : k * 512 + (j + 1) * 128],
                        rhs=xt_sb[:, k * T2 + nb * 512 : k * T2 + (nb + 1) * 512],
                        start=(k == 0),
                        stop=(k == KT - 1),
                    )
                nc.vector.tensor_scalar_add(
                    xw_sb[:, j * T2 + nb * 512 : j * T2 + (nb + 1) * 512],
                    ps[:],
                    b_sb[:, j : j + 1],
                )

            def lstm_step(hist, hprev_ap, xw_sb, whh_sb, t):
                pg = pgate.tile([128, 2048], DT.float32, tag="g")
                for j in range(4):
                    for k in range(KT):
                        nc.tensor.matmul(
                            pg[:, j * 512 : j * 512 + B],
                            lhsT=whh_sb[:, k * 512 + j * 128 : k * 512 + (j + 1) * 128],
                            rhs=hprev_ap[:, k * B : (k + 1) * B],
                            start=(k == 0),
                            stop=(k == KT - 1),
                        )
                g_in = wk.tile([128, 64], DT.float32, tag="gin")
                nc.vector.tensor_add(
                    g_in[:].rearrange("p (j b) -> p j b", j=4),
                    pg[:].rearrange("p (j x) -> p j x", j=4)[:, :, 0:B],
                    xw_sb[:].rearrange("p (j n) -> p j n", j=4)[:, :, t * B : (t + 1) * B],
                )
                a = wk.tile([128, 64], DT.float32, tag="act")
                nc.scalar.activation(a[:, 0:48], g_in[:, 0:48], AF.Sigmoid)
                nc.scalar.activation(a[:, 48:64], g_in[:, 48:64], AF.Tanh)
                m1 = wk.tile([128, B], DT.float32, tag="m1")
                m2 = wk.tile([128, B], DT.float32, tag="m2")
                nc.vector.tensor_mul(m1[:], a[:, 0:16], c_sb[:])
                nc.vector.tensor_mul(m2[:], a[:, 16:32], a[:, 48:64])
                nc.vector.tensor_add(c_sb[:], m1[:], m2[:])
                tct = wk.tile([128, B], DT.float32, tag="tct")
                nc.scalar.activation(tct[:], c_sb[:], AF.Tanh)
                h_sb = wk.tile([128, B], DT.bfloat16, tag="h")
                nc.vector.tensor_mul(h_sb[:], a[:, 32:48], tct[:])
                # exchange: AllGather h slices -> full h in hist[:, t*128:(t+1)*128]
                if CC_MODE == "sbuf":
                    hs = h_sb[:]
                    nc.sync.dma_start(
                        hist[:, t * 128 : (t + 1) * 128].rearrange("p (k b) -> p k b", k=KT),
                        bass.AP(hs.tensor, hs.offset, [list(hs.ap[0]), [0, KT], [1, B]]),
                    )
                    return
                cin = dcc.tile([128, B], DT.bfloat16, tag="cin")
                cout = dcc.tile([NC * 128, B], DT.bfloat16, tag="cout")
                nc.sync.dma_start(cin[:], h_sb[:])
                if CC_MODE == "cc":
                    nc.gpsimd.collective_compute(
                        "AllGather",
                        ALU.bypass,
                        replica_groups=[list(range(NC))],
                        ins=[cin[:].opt()],
                        outs=[cout[:].opt()],
                    )
                else:  # "dram"
                    ci = cin[:]
                    nc.sync.dma_start(
                        cout[:].rearrange("(k p) b -> k p b", p=128),
                        bass.AP(ci.tensor, ci.offset, [[0, KT], [B, 128], [1, B]]),
                    )
                nc.sync.dma_start(
                    hist[:, t * 128 : (t + 1) * 128].rearrange("p (k b) -> p k b", k=KT),
                    cout[:].rearrange("(k p) b -> p k b", p=128),
                )

            # ================= background task machinery =================
            def fc_chunk(mj, nb):
                f1t = ws.tile([128, KT * 128], DT.bfloat16, tag="f1t")
                nc.sync.dma_start(f1t[:], fw1[:, mj * 1024 : (mj + 1) * 1024])
                ps = paux.tile([128, 512], DT.float32, tag="aux")
                hv = hist_d[:].rearrange("p (t k b) -> p t k b", t=n_steps, k=KT)
                for k in range(KT):
                    nc.tensor.matmul(
                        ps[:],
                        lhsT=f1t[:, k * 128 : (k + 1) * 128],
                        rhs=hv[:, nb * 32 : (nb + 1) * 32, k : k + 1, :],
                        start=(k == 0),
                        stop=(k == KT - 1),
                    )
                fco = ws.tile([128, 512], DT.float32, tag="fco")
                b2 = bias2_sb[:, mj * B : (mj + 1) * B]  # [128, 16]
                nc.vector.tensor_tensor(
                    fco[:].rearrange("p (t b) -> p t b", t=32),
                    ps[:].rearrange("p (t b) -> p t b", t=32),
                    _bcast(b2, 1, 32),  # [128, 32(bcast), 16]
                    op=ALU.add,
                )
                nc.sync.dma_start(
                    out[mj * 128 : (mj + 1) * 128, nb * 512 : (nb + 1) * 512], fco[:]
                )

            def se_quarter(q):
                ps = paux.tile([1, 512], DT.float32, tag="aux")
                hv = hist_e[:].rearrange("p (t k b) -> p t k b", t=n_steps, k=KT)
                for k in range(KT):
                    nc.tensor.matmul(
                        ps[:],
                        lhsT=wet_sb[:, k : k + 1],
                        rhs=hv[:, q * 32 : (q + 1) * 32, k : k + 1, :],
                        start=(k == 0),
                        stop=(k == KT - 1),
                    )
                nc.scalar.activation(se_sb[:, q * 512 : (q + 1) * 512], ps[:], AF.Identity)

            se_dram = dcc.tile([1, T2], DT.float32, tag="sed")
            abc_dram = dcc.tile([B, n_steps], DT.bfloat16, tag="abcd")

            def softmax_pieces():
                # emitted over several steps to hide DMA latency
                yield lambda: nc.sync.dma_start(se_dram[:], se_sb[:])
                # se layout: free = e*16 + b; load transposed [b, e]
                yield lambda: nc.sync.dma_start(
                    seT_sb[:],
                    bass.AP(se_dram[:].tensor, se_dram[:].offset, [[1, B], [B, n_steps]]),
                )

                def red():
                    mx = wk.tile([B, 1], DT.float32, tag="mx")
                    nc.vector.reduce_max(mx[:], seT_sb[:], axis=mybir.AxisListType.X)
                    nmx = wk.tile([B, 1], DT.float32, tag="nmx")
                    nc.vector.tensor_scalar(nmx[:], mx[:], -1.0, None, op0=ALU.mult)
                    ex = wk.tile([B, n_steps], DT.float32, tag="ex")
                    nc.scalar.activation(ex[:], seT_sb[:], AF.Exp, bias=nmx[:])
                    sm = wk.tile([B, 1], DT.float32, tag="sm")
                    nc.vector.reduce_sum(sm[:], ex[:], axis=mybir.AxisListType.X)
                    rs = wk.tile([B, 1], DT.float32, tag="rs")
                    nc.vector.reciprocal(rs[:], sm[:])
                    nc.vector.tensor_scalar(attn_sb[:], ex[:], rs[:], None, op0=ALU.mult)

                yield red
                yield lambda: nc.sync.dma_start(abc_dram[:], attn_sb[:])
                # broadcast [B*n_steps] to all 128 partitions
                yield lambda: nc.sync.dma_start(
                    abc_sb[:],
                    bass.AP(abc_dram[:].tensor, abc_dram[:].offset, [[0, 128], [1, B * n_steps]]),
                )

            def ctx_chunk(k, bh):
                # half bh of batches for h-tile k: ctx[k*16+b] = sum_e hist_e * attn
                tmp = wk.tile([128, 8 * n_steps], DT.float32, tag="ctmp")
                he = hist_e[:].rearrange("p (e k b) -> p k b e", e=n_steps, k=KT)
                ab = abc_sb[:].rearrange("p (b e) -> p b e", b=B)
                b0 = bh * 8
                nc.vector.tensor_tensor(
                    tmp[:].rearrange("p (b e) -> p b e", b=8),
                    he[:, k, b0 : b0 + 8, :],
                    ab[:, b0 : b0 + 8, :],
                    op=ALU.mult,
                )
                ctf = wk.tile([128, 8], DT.float32, tag="ctf")
                nc.vector.reduce_sum(
                    ctf[:],
                    tmp[:].rearrange("p (b e) -> p b e", b=8),
                    axis=mybir.AxisListType.X,
                )
                nc.vector.tensor_copy(ctx_sb[:, k * B + b0 : k * B + b0 + 8], ctf[:])

            def bias2_chunk(mj):
                f2t = ws.tile([128, KT * 128], DT.bfloat16, tag="f2t")
                nc.sync.dma_start(f2t[:], fw2[:, mj * 1024 : (mj + 1) * 1024])
                ps = paux.tile([128, B], DT.float32, tag="aux")
                for k in range(KT):
                    nc.tensor.matmul(
                        ps[:],
                        lhsT=f2t[:, k * 128 : (k + 1) * 128],
                        rhs=ctx_sb[:, k * B : (k + 1) * B],
                        start=(k == 0),
                        stop=(k == KT - 1),
                    )
                nc.scalar.activation(
                    bias2_sb[:, mj * B : (mj + 1) * B],
                    ps[:],
                    AF.Identity,
                    bias=fcb_sb[:, mj : mj + 1],
                )

            # ================= emission =================
            load_xt(xet)
            for j in range(4):
                for nb in range(4):
                    xw_chunk(xwe_sb, wie_sb, be_sb, j, nb)

            # encoder phase; background: load x_dec, compute xw_dec
            enc_bg = [lambda s=src: load_xt(s) for src in (xdt,)]
            enc_bg += [
                (lambda j=j, nb=nb: xw_chunk(xwd_sb, wid_sb, bd_sb, j, nb))
                for j in range(4)
                for nb in range(4)
            ]
            for t in range(n_steps):
                hprev = h0_sb[:] if t == 0 else hist_e[:, (t - 1) * 128 : t * 128]
                lstm_step(hist_e, hprev, xwe_sb, whe_sb, t)
                if t >= 2 and t % 6 == 2 and enc_bg:
                    enc_bg.pop(0)()

            # decoder phase; background: attention, bias2, then fc chunks
            dec_bg = [(lambda q=q: se_quarter(q)) for q in range(4)]
            dec_bg += list(softmax_pieces())
            dec_bg += [
                (lambda k=k, bh=bh: ctx_chunk(k, bh)) for k in range(KT) for bh in range(2)
            ]
            dec_bg += [(lambda mj=mj: bias2_chunk(mj)) for mj in range(MT)]
            fc_ready = {nb: 32 * (nb + 1) + 1 for nb in range(4)}
            fc_tasks = [(nb, mj) for nb in range(4) for mj in range(MT)]
            fc_i = 0
            for t in range(n_steps):
                hprev = (
                    hist_e[:, (n_steps - 1) * 128 : n_steps * 128]
                    if t == 0
                    else hist_d[:, (t - 1) * 128 : t * 128]
                )
                lstm_step(hist_d, hprev, xwd_sb, whd_sb, t)
                if t >= 1 and dec_bg:
                    dec_bg.pop(0)()
                    if dec_bg:
                        dec_bg.pop(0)()
                elif fc_i < len(fc_tasks) and t >= fc_ready[fc_tasks[fc_i][0]]:
                    nb, mj = fc_tasks[fc_i]
                    fc_chunk(mj, nb)
                    fc_i += 1
            while fc_i < len(fc_tasks):
                nb, mj = fc_tasks[fc_i]
                fc_chunk(mj, nb)
                fc_i += 1


# ---------------- host side ----------------


def _gate_rows(m):
    return np.concatenate(
        [np.arange(g * H + m * HL, g * H + (m + 1) * HL) for g in GATE_SRC]
    )


def _pack_whh(w, rows):
    """[4H, H] weight -> per-core [128, KT*512] bf16 sbuf layout (k, j, c)."""
    lhsT = np.ascontiguousarray(w[rows].T)  # [1024, 512]
    return (
        lhsT.reshape(KT, 128, 4, 128).transpose(1, 0, 2, 3).reshape(128, KT * 512)
    ).astype(BF16)


def _pack_fc(wpart):
    """[4096, 1024] -> [128, MT*KT*128] bf16 layout (mj, k, c)."""
    lhsT = np.ascontiguousarray(wpart.T)  # [1024, 4096]
    return (
        lhsT.reshape(KT, 128, MT, 128).transpose(1, 2, 0, 3).reshape(128, MT * KT * 128)
    ).astype(BF16)


def _xT(emb_rows):
    """[B, T, H] f32 -> [H, T2] bf16 with tau = t*B + b."""
    xt = np.transpose(emb_rows, (1, 0, 2)).reshape(T2, H)
    return np.ascontiguousarray(xt.T).astype(BF16)


_NC_CACHE = {}


def _get_nc():
    if "nc" not in _NC_CACHE:
        _NC_CACHE["nc"] = build_nc()
    return _NC_CACHE["nc"]


def make_in_maps(
    src, tgt, src_emb, tgt_emb, enc_Wih, enc_Whh, enc_bih, enc_bhh,
    dec_Wih, dec_Whh, dec_bih, dec_bhh, attn_w, attn_b, fc_w, fc_b,
):
    src = np.asarray(src)
    tgt = np.asarray(tgt)
    xet = _xT(np.asarray(src_emb, np.float32)[src])
    xdt = _xT(np.asarray(tgt_emb, np.float32)[tgt])
    b_enc = np.asarray(enc_bih, np.float32) + np.asarray(enc_bhh, np.float32)
    b_dec = np.asarray(dec_bih, np.float32) + np.asarray(dec_bhh, np.float32)
    we = np.asarray(attn_w, np.float32)[0, H:]
    wet = np.ascontiguousarray(we.reshape(KT, 128).T).astype(BF16)
    fc_w = np.asarray(fc_w, np.float32)
    fc_b = np.asarray(fc_b, np.float32)

    in_maps = []
    for m in range(NC):
        rows = _gate_rows(m)
        vlo = m * VL
        wrows = np.zeros((VLP, 2 * H), np.float32)
        nreal = min(VLP, V - vlo)
        wrows[:nreal] = fc_w[vlo : vlo + nreal]
        brows = np.zeros((VLP,), np.float32)
        brows[:nreal] = fc_b[vlo : vlo + nreal]
        in_maps.append(
            {
                "xet": xet,
                "xdt": xdt,
                "whe": _pack_whh(np.asarray(enc_Whh, np.float32), rows),
                "wie": _pack_whh(np.asarray(enc_Wih, np.float32), rows),
                "whd": _pack_whh(np.asarray(dec_Whh, np.float32), rows),
                "wid": _pack_whh(np.asarray(dec_Wih, np.float32), rows),
                "be": np.ascontiguousarray(b_enc[rows].reshape(4, 128).T),
                "bd": np.ascontiguousarray(b_dec[rows].reshape(4, 128).T),
                "fw1": _pack_fc(wrows[:, :H]),
                "fw2": _pack_fc(wrows[:, H:]),
                "fcb": np.ascontiguousarray(brows.reshape(MT, 128).T),
                "wet": wet,
            }
        )
    return in_maps


def kernel(**inputs):
    nc = _get_nc()
    in_maps = make_in_maps(**inputs)
    res = run_bass_kernel_spmd(nc, in_maps, core_ids=list(range(NC)))
    shards = [np.asarray(r["out"], np.float32)[:VL] for r in res.results]
    full = np.concatenate(shards, axis=0)  # [V, T2]
    return np.ascontiguousarray(full.reshape(V, T, B).transpose(2, 1, 0))
